# revision 7
# baseline (speedup 1.0000x reference)
"""BinaryExceptOutliersLinear on 8 Trainium2 NeuronCores.

Reference computation:
    w_bin = where(|w - mean(w)| > std(w), w, sign(w))   (mean/std over all of w, ddof=1)
    out[b,s,o] = sum_k x[b,s,k] * w_bin[o,k] + bias[o]

Strategy (data-parallel over tokens):
  - The batch dim B=8 is sharded across the 8 cores (2048 tokens each);
    every core gets the full weight + bias and computes its tokens' full
    output row-block.  No collectives needed.
  - The binarization thresholds (mean/std of w) are computed host-side with
    jax-on-CPU using the exact op sequence of the reference, so the outlier
    decision boundary matches the grader's reference bit-for-bit.  The
    binarize itself (clamp/compare/select + sign) runs on-device.
  - On-core pipeline per output-row tile (128 rows of w):
      DMA w rows -> binarize (ACT sign + DVE clamp/neq/copy_predicated,
      output bf16) -> PE-transpose into w^T[k,o] layout -> 32x k-matmuls
      against a resident bf16 x^T[k,t] (built once by a PE-transpose
      prepass) accumulating in 4 PSUM banks (512 tokens each) -> DVE adds
      bias while copying PSUM->SBUF -> DMA out.
  - Output is produced transposed ([d_out, tokens] per core) so PSUM
    partitions carry d_out (bias is per-partition); the host transposes
    back when unsharding.
"""

import os
import sys

import numpy as np

for _p in ("/opt/trn_rl_repo", "/opt/pypackages"):
    if os.path.isdir(_p) and _p not in sys.path:
        sys.path.append(_p)

P = 128
B, S, D_IN, D_OUT = 8, 2048, 4096, 4096
N_CORES = 8
T = (B * S) // N_CORES  # tokens per core = 2048

F32 = None  # filled lazily (mybir import is heavy)
BF16 = None


def build_program(
    t=T,
    d_in=D_IN,
    d_out=D_OUT,
    t_tile=512,
    k_chunk=1024,
    enable_asserts=False,
):
    """Build the single-core Bass/Tile program (same program runs on all cores)."""
    import concourse.mybir as mybir
    import concourse.tile as tile
    from concourse import bacc

    global F32, BF16
    F32 = mybir.dt.float32
    BF16 = mybir.dt.bfloat16
    AF = mybir.ActivationFunctionType
    ALU = mybir.AluOpType

    assert t % t_tile == 0 and d_in % P == 0 and d_out % P == 0
    assert d_in % k_chunk == 0 and k_chunk % P == 0

    KSUB = d_in // P          # k subtiles of 128
    T_TILES = t // t_tile     # psum banks used for accumulation
    O_TILES = d_out // P
    KC_PER = k_chunk // P     # k subtiles per binarize chunk
    N_CHUNKS = d_in // k_chunk

    nc = bacc.Bacc(
        "TRN2",
        target_bir_lowering=False,
        debug=False,
        enable_asserts=enable_asserts,
        num_devices=1,
    )

    x = nc.dram_tensor("x", [t, d_in], F32, kind="ExternalInput").ap()
    w = nc.dram_tensor("w", [d_out, d_in], F32, kind="ExternalInput").ap()
    bias = nc.dram_tensor("bias", [d_out], F32, kind="ExternalInput").ap()
    thr = nc.dram_tensor("thr", [P, 2], F32, kind="ExternalInput").ap()
    outT = nc.dram_tensor("outT", [d_out, t], F32, kind="ExternalOutput").ap()

    from concourse.masks import make_identity

    with tile.TileContext(nc) as tc:
        with (
            tc.tile_pool(name="const", bufs=1) as const,
            tc.tile_pool(name="psum_acc", bufs=T_TILES, space="PSUM") as psum_acc,
        ):
            ident = const.tile([P, P], BF16)
            make_identity(nc, ident)

            bias_sb = const.tile([P, O_TILES], F32)
            nc.sync.dma_start(bias_sb, bias.rearrange("(o p) -> p o", p=P))
            thr_sb = const.tile([P, 2], F32)
            nc.sync.dma_start(thr_sb, thr)
            lower = thr_sb[:, 0:1]
            upper = thr_sb[:, 1:2]

            # x^T resident in SBUF as bf16: [128(k), KSUB, t]
            xT = const.tile([P, KSUB, t], BF16)

            # ---- prepass: x -> bf16 -> PE-transpose -> xT ----
            PGRP = 2  # token-panels per group (batches psum->sbuf copies)
            with (
                tc.tile_pool(name="xpre", bufs=2) as xpre,
                tc.tile_pool(name="xpre_bf", bufs=2 * PGRP) as xpre_bf,
                tc.tile_pool(name="psum_x", bufs=2, space="PSUM") as psum_x,
            ):
                t_panels = t // P
                for tg in range(t_panels // PGRP):
                    xbfs = []
                    for pi in range(PGRP):
                        tp = tg * PGRP + pi
                        xraw = xpre.tile([P, d_in], F32)
                        nc.sync.dma_start(xraw, x[tp * P : (tp + 1) * P, :])
                        xbf = xpre_bf.tile([P, d_in], BF16)
                        nc.vector.tensor_copy(xbf, xraw)
                        xbfs.append(xbf)
                    for ks in range(KSUB):
                        pt = psum_x.tile([P, PGRP * P], BF16)
                        for pi in range(PGRP):
                            nc.tensor.transpose(
                                pt[:, pi * P : (pi + 1) * P],
                                xbfs[pi][:, ks * P : (ks + 1) * P],
                                ident,
                            )
                        nc.scalar.activation(
                            xT[:, ks, tg * PGRP * P : (tg + 1) * PGRP * P],
                            pt,
                            AF.Copy,
                        )

            # ---- main loop over output-row tiles ----
            with (
                tc.tile_pool(name="wraw", bufs=2) as wraw_pool,
                tc.tile_pool(name="wmask", bufs=1) as wmask_pool,
                tc.tile_pool(name="wbin", bufs=2) as wbin_pool,
                tc.tile_pool(name="wT", bufs=1) as wT_pool,
                tc.tile_pool(name="osb", bufs=2) as osb_pool,
                tc.tile_pool(name="psum_t", bufs=2, space="PSUM") as psum_t,
            ):
                for ot in range(O_TILES):
                    # stage + binarize w rows [128, d_in] in k-chunks
                    wT_col = wT_pool.tile([P, KSUB, P], BF16)
                    for ch in range(N_CHUNKS):
                        wraw = wraw_pool.tile([P, k_chunk], F32)
                        nc.sync.dma_start(
                            wraw,
                            w[ot * P : (ot + 1) * P, ch * k_chunk : (ch + 1) * k_chunk],
                        )
                        wbin = wbin_pool.tile([P, k_chunk], BF16)
                        # sign(w) -> bf16 (exact +-1/0)
                        nc.scalar.activation(wbin, wraw, AF.Sign)
                        # outlier mask: clamp(w) != w  (mask must be int dtype
                        # for CopyPredicated on hardware)
                        wm = wmask_pool.tile([P, k_chunk], F32)
                        nc.vector.tensor_scalar(
                            wm, wraw, lower, upper, ALU.max, ALU.min
                        )
                        wmask = wmask_pool.tile([P, k_chunk], mybir.dt.uint8)
                        nc.vector.tensor_tensor(wmask, wm, wraw, ALU.not_equal)
                        # outliers keep original value (cast to bf16 on write)
                        nc.vector.copy_predicated(wbin, wmask, wraw)
                        # PE-transpose each 128x128 block into wT_col
                        for kc in range(KC_PER):
                            ks = ch * KC_PER + kc
                            pt = psum_t.tile([P, P], BF16)
                            nc.tensor.transpose(
                                pt, wbin[:, kc * P : (kc + 1) * P], ident
                            )
                            nc.scalar.activation(wT_col[:, ks, :], pt, AF.Copy)

                    # matmuls: psum[ot, tt] += wT_col[:,ks,:].T @ xT[:,ks,tt]
                    psums = [
                        psum_acc.tile([P, t_tile], F32, name=f"acc{tt}", tag="acc")
                        for tt in range(T_TILES)
                    ]
                    for ks in range(KSUB):
                        for tt in range(T_TILES):
                            nc.tensor.matmul(
                                psums[tt],
                                wT_col[:, ks, :],
                                xT[:, ks, tt * t_tile : (tt + 1) * t_tile],
                                start=(ks == 0),
                                stop=(ks == KSUB - 1),
                            )

                    # psum -> sbuf with bias add, then DMA out
                    osb = osb_pool.tile([P, t], F32)
                    for tt in range(T_TILES):
                        nc.vector.tensor_scalar(
                            osb[:, tt * t_tile : (tt + 1) * t_tile],
                            psums[tt],
                            bias_sb[:, ot : ot + 1],
                            None,
                            ALU.add,
                        )
                    nc.sync.dma_start(outT[ot * P : (ot + 1) * P, :], osb)

    nc.compile()
    return nc


def _thresholds(weight):
    """Replicate the reference's threshold computation bit-exactly (jax CPU fp32)."""
    import jax
    import jax.numpy as jnp

    cpu = jax.devices("cpu")[0]
    with jax.default_device(cpu):
        wj = jnp.asarray(weight)
        mean = jnp.mean(wj)
        std = jnp.std(wj, ddof=1)
        lower = np.float32(np.asarray(mean - std))
        upper = np.float32(np.asarray(mean + std))
    return lower, upper


_PROGRAM_CACHE = {}


def kernel(x, weight, bias):
    from concourse.bass_utils import run_bass_kernel_spmd

    assert x.shape == (B, S, D_IN) and weight.shape == (D_OUT, D_IN)
    x = np.ascontiguousarray(np.asarray(x, dtype=np.float32))
    weight = np.ascontiguousarray(np.asarray(weight, dtype=np.float32))
    bias = np.ascontiguousarray(np.asarray(bias, dtype=np.float32))

    lower, upper = _thresholds(weight)
    thr = np.tile(np.array([[lower, upper]], dtype=np.float32), (P, 1))

    if "full" not in _PROGRAM_CACHE:
        _PROGRAM_CACHE["full"] = build_program()
    nc = _PROGRAM_CACHE["full"]

    x_sh = x.reshape(N_CORES, T, D_IN)
    in_maps = [
        {"x": x_sh[i], "w": weight, "bias": bias, "thr": thr} for i in range(N_CORES)
    ]
    res = run_bass_kernel_spmd(nc, in_maps, core_ids=list(range(N_CORES)))
    out = np.empty((N_CORES, T, D_OUT), dtype=np.float32)
    for i in range(N_CORES):
        out[i] = res.results[i]["outT"].T
    return out.reshape(B, S, D_OUT)
